# revision 43
# baseline (speedup 1.0000x reference)
"""DeChunk EMA-scan kernel for Trainium2 (Bass/Tile), 8 NeuronCores.

Problem: out[b,t,:] = p_t * x_t + (1-p_t) * out[b,t-1,:], where
x_t = hidden[b, idx_t, :], idx = cumsum(boundary_mask)-1,
p = clip(boundary_prob[...,1], EPS, 1-EPS) with p[:,0]=1.

Sharding: pure data parallel. core c handles batch b=c//2 and channel half
dh=c%2 (512 of 1024 channels). No cross-core communication.

Device algorithm per core (L=8192, Dc=512):
  - idx = global cumsum(mask)-1 via triangular matmuls (tile-local cumsum +
    tile-offset fixup), converted to the int16 "16-partition wrapped" layout
    dma_gather wants.
  - gathered = hid[idx] via dma_gather in 8 chunks, landing tile-major:
    position t -> partition t%128, free block t//128.
  - Blocked scan over 64 tiles of T=128: out_tile = Mk @ (p*x) + A (x) carry,
    where Mk[t,s] = prod_{r=s+1..t} a_r * p_s = exp(S_t - S_s + log p_s)
    (tile-local S = cumsum log a), built with one K=1 matmul (broadcast S_t
    down partitions), a mask add (-3e38 upper part), and one ScalarE exp
    with per-partition bias (log p_s - S_s). A_t = exp(S_t).
  - Carry c_k = out_k[127,:] feeds tile k+1 via a K=1 bf16 matmul accumulated
    into the same PSUM bank as the big matmul.
"""

import sys

for _p in ("/opt/trn_rl_repo", "/root/.axon_site/_ro/trn_rl_repo"):
    if _p not in sys.path:
        sys.path.insert(0, _p)

import numpy as np
from contextlib import ExitStack

import concourse.bass as bass
import concourse.tile as tile
from concourse import bacc, mybir
from concourse._compat import with_exitstack

B, L, D = 4, 8192, 1024
N_CORES = 8
DC = D // 2  # channels per core
T = 128  # scan tile length
EPS = 1e-4
F32 = mybir.dt.float32
BF16 = mybir.dt.bfloat16
I16 = mybir.dt.int16
ALU = mybir.AluOpType
ACTF = mybir.ActivationFunctionType


@with_exitstack
def _dechunk_tile_kernel(
    ctx: ExitStack,
    tc: "tile.TileContext",
    out_ap: bass.AP,
    hid_ap: bass.AP,
    p_ap: bass.AP,
    m_ap: bass.AP,
    triu_ap: bass.AP,
    mbias_ap: bass.AP,
    ident_ap: bass.AP,
    Lk: int,
    Dk: int,
    n_gather_chunks: int,
    mm_mode: str = "f32r",
):
    nc = tc.nc
    F32R = mybir.dt.float32r
    # mm_mode: "f32r" = float32r big/fixup matmuls (1 cycle/row, ~17-bit
    # mantissa products); "f32" = plain fp32 matmuls (4 cycles/row, safest);
    # "bf16" = fp32 big matmul + bf16 carry/fixup (~2e-3 absmax err).
    assert mm_mode in ("f32r", "f32", "bf16", "bf16g")
    # "bf16g": hidden states shipped/gathered as bf16, M-matrix bf16 — big
    # matmul at true 1 cycle/row and half the gather bytes (~4e-3 absmax).
    mm_dt = {"f32r": F32R, "bf16g": BF16}.get(mm_mode, F32)
    # carry path must avoid float32r (DVE has no f32r encodings); bf16 keeps
    # the fixup matmul at 1 cycle/row and costs ~2e-3 absmax error.
    carry_dt = F32 if mm_mode == "f32" else BF16
    nt = Lk // T  # number of scan tiles
    assert Lk % (16 * n_gather_chunks) == 0
    ch_idx = Lk // n_gather_chunks  # indices per gather chunk
    assert ch_idx % T == 0
    ch_tiles = ch_idx // T  # scan tiles per gather chunk

    STG = 4  # output tiles batched per store DMA
    assert nt % STG == 0

    const = ctx.enter_context(tc.tile_pool(name="const", bufs=1))
    sb = ctx.enter_context(tc.tile_pool(name="sb", bufs=1))
    gat_pool = ctx.enter_context(tc.tile_pool(name="gat", bufs=n_gather_chunks))
    lhs_pool = ctx.enter_context(tc.tile_pool(name="lhs", bufs=4))
    stage_pool = ctx.enter_context(tc.tile_pool(name="stage", bufs=4))
    outsb_pool = ctx.enter_context(tc.tile_pool(name="outsb", bufs=2))
    crow_pool = ctx.enter_context(tc.tile_pool(name="crow", bufs=6))
    psum_small = ctx.enter_context(tc.tile_pool(name="psum_sm", bufs=2, space="PSUM"))
    psum_scol = ctx.enter_context(tc.tile_pool(name="psum_scol", bufs=2, space="PSUM"))
    psum_out = ctx.enter_context(tc.tile_pool(name="psum_out", bufs=4, space="PSUM"))

    # ---- load constants / small inputs
    ctri = const.tile([T, T], F32)
    nc.sync.dma_start(out=ctri[:], in_=triu_ap)
    cmb = const.tile([T, T], F32)
    nc.sync.dma_start(out=cmb[:], in_=mbias_ap)
    cid = const.tile([T, T], F32)
    nc.sync.dma_start(out=cid[:], in_=ident_ap)
    p_sb = const.tile([T, nt], F32)
    nc.sync.dma_start(out=p_sb[:], in_=p_ap)
    m16_sb = const.tile([16, Lk // 16], F32)
    nc.sync.dma_start(out=m16_sb[:], in_=m_ap)

    # ---- p processing: clip, p0=1, a=1-p (a0=tiny), logs
    pc = sb.tile([T, nt], F32, tag="pc")
    nc.vector.tensor_scalar(
        out=pc[:], in0=p_sb[:], scalar1=EPS, scalar2=1.0 - EPS, op0=ALU.max, op1=ALU.min
    )
    nc.vector.memset(pc[0:1, 0:1], 1.0)
    av = sb.tile([T, nt], F32, tag="av")
    nc.vector.tensor_scalar(
        out=av[:], in0=pc[:], scalar1=-1.0, scalar2=1.0, op0=ALU.mult, op1=ALU.add
    )
    nc.vector.memset(av[0:1, 0:1], 1e-30)
    loga = sb.tile([T, nt], F32, tag="loga")
    nc.scalar.activation(loga[:], av[:], ACTF.Ln)
    logp = sb.tile([T, nt], F32, tag="logp")
    nc.scalar.activation(logp[:], pc[:], ACTF.Ln)

    # ---- S = tile-local inclusive cumsum of log a (one matmul for all tiles)
    S_ps = psum_small.tile([T, nt], F32, tag="small")
    nc.tensor.matmul(S_ps[:], lhsT=ctri[:], rhs=loga[:], start=True, stop=True)
    S_sb = sb.tile([T, nt], F32, tag="S_sb")
    nc.vector.tensor_copy(S_sb[:], S_ps[:])
    biasv = sb.tile([T, nt], F32, tag="biasv")
    nc.vector.tensor_tensor(out=biasv[:], in0=logp[:], in1=S_sb[:], op=ALU.subtract)
    # S_T[k, t] = S_sb[t, k], rotated (column t' holds position (t'-1)%T) and
    # flattened to a single-partition row S_flat[0, k*T + t'] so per-tile S
    # rows are free-dim slices (engine APs may only start at partitions
    # 0/32/64/96).
    S_T_ps = psum_small.tile([nt, T], F32, tag="small")
    nc.tensor.transpose(S_T_ps[:], S_sb[:], cid[:])
    S_T_rot = sb.tile([nt, T], F32, tag="S_T_rot")
    nc.vector.tensor_copy(S_T_rot[:, 1:T], S_T_ps[:, 0 : T - 1])
    nc.vector.tensor_copy(S_T_rot[:, 0:1], S_T_ps[:, T - 1 : T])
    S_flat = sb.tile([1, nt * T], F32, tag="S_flat")
    nc.sync.dma_start(
        out=S_flat[:].rearrange("p (k t) -> p k t", t=T), in_=S_T_rot[:]
    )

    # ---- idx = global cumsum(mask) - 1, directly in the int16
    # "16-partition wrapped" layout dma_gather wants: idx16[q, s] = idx[s*16+q].
    ns = Lk // 16
    zeros16 = sb.tile([16, ns], F32, tag="zeros16")
    nc.vector.memset(zeros16[:], 0.0)
    # csw[q, s] = sum_{s'<=s} m16[q, s']  (per-partition running sum)
    csw = sb.tile([16, ns], F32, tag="csw")
    nc.vector.tensor_tensor_scan(
        out=csw[:], data0=m16_sb[:], data1=zeros16[:], initial=0.0,
        op0=ALU.add, op1=ALU.add,
    )
    # totcum[s] = total mask count through column s (sum over the 16 rows)
    totcum_ps = psum_small.tile([1, ns], F32, tag="small")
    nc.tensor.matmul(
        totcum_ps[:], lhsT=ctri[0:16, T - 1 : T], rhs=csw[:], start=True, stop=True
    )
    # offs16[s] = totcum[s-1], offs16[0] = 0
    offs16 = sb.tile([1, ns], F32, tag="offs16")
    nc.vector.memset(offs16[0:1, 0:1], 0.0)
    nc.vector.tensor_copy(offs16[0:1, 1:ns], totcum_ps[0:1, 0 : ns - 1])
    # idx16_ps[q, s] = (within-column cumsum)[q, s] + offs16[s]
    idx16_ps = psum_small.tile([16, ns], F32, tag="small")
    nc.tensor.matmul(
        idx16_ps[:], lhsT=ctri[0:16, 0:16], rhs=m16_sb[:], start=True, stop=False
    )
    nc.tensor.matmul(
        idx16_ps[:], lhsT=ctri[0:1, 0:16], rhs=offs16[:], start=False, stop=True
    )
    idx16 = sb.tile([T, ns], I16, tag="idx16")
    nc.vector.tensor_scalar(
        out=idx16[0:16, :], in0=idx16_ps[:], scalar1=-1.0, scalar2=None, op0=ALU.add
    )
    # replicate to all 8 partition groups (one per Q7 core) via SBUF->SBUF DMA
    nc.sync.dma_start(out=idx16[16:32, :], in_=idx16[0:16, :])
    nc.sync.dma_start(out=idx16[32:64, :], in_=idx16[0:32, :])
    nc.sync.dma_start(out=idx16[64:128, :], in_=idx16[0:64, :])

    # ---- gather: hid[idx] in chunks, tile-major layout
    gat_tiles = []
    for c in range(n_gather_chunks):
        g_t = gat_pool.tile([T, ch_tiles * Dk], mm_dt, tag="gat")
        g3 = g_t[:].rearrange("p (j d) -> p j d", d=Dk)
        nc.gpsimd.dma_gather(
            out_ap=g3,
            in_ap=hid_ap.bitcast(mm_dt),
            idxs_ap=idx16[:, c * (ch_idx // 16) : (c + 1) * (ch_idx // 16)],
            num_idxs=ch_idx,
            num_idxs_reg=ch_idx,
            elem_size=Dk,
        )
        gat_tiles.append(g_t)

    # ---- main blocked scan
    # Output rows are ROTATED by one: out partition t' holds position
    # (t'-1) % T, so the carry row (position T-1) lands at partition 0,
    # which is the only legal engine-read start. The host unpermutes.
    SCH = 4  # tiles per Scol-broadcast chunk (N = SCH*T = 512 per matmul)
    QP = 2  # software pipeline depth: bigmm runs QP tiles ahead of fixup
    assert nt % SCH == 0
    carry = {}  # k -> carry tile holding c_k
    a_rows = {}
    ops_tiles = {}
    osb_tiles = {}

    def emit_front(k):
        # lhsT build + big matmul for tile k (independent of the carry chain)
        if k % SCH == 0:
            tmp_ps = psum_scol.tile([T, SCH * T], F32, tag="scol")
            nc.tensor.matmul(
                tmp_ps[:],
                lhsT=ctri[0:1, :],
                rhs=S_flat[0:1, k * T : (k + SCH) * T],
                start=True,
                stop=True,
            )
            emit_front.tmp_ps = tmp_ps
        tmp_ps = emit_front.tmp_ps
        j = k % SCH
        nc.vector.tensor_tensor(
            out=tmp_ps[:, j * T : (j + 1) * T],
            in0=tmp_ps[:, j * T : (j + 1) * T],
            in1=cmb[:],
            op=ALU.add,
        )
        lhsT_k = lhs_pool.tile([T, T], mm_dt, tag="lhsT")
        nc.scalar.activation(
            lhsT_k[:],
            tmp_ps[:, j * T : (j + 1) * T],
            ACTF.Exp,
            bias=biasv[:, k : k + 1],
            scale=1.0,
        )
        if k > 0:
            a_row = stage_pool.tile([1, T], carry_dt, tag="a_row")
            nc.scalar.activation(
                a_row[:], S_flat[0:1, k * T : (k + 1) * T], ACTF.Exp
            )
            a_rows[k] = a_row
        ops = psum_out.tile([T, Dk], F32, tag="ops")
        rhs = gat_tiles[k // ch_tiles][:].rearrange("p (j d) -> p j d", d=Dk)[
            :, k % ch_tiles, :
        ]
        nc.tensor.matmul(ops[:], lhsT=lhsT_k[:], rhs=rhs, start=True, stop=True)
        ops_tiles[k] = ops

        # Fast serial carry: c_k = A_k[T-1] * c_{k-1} + ops0[row 0]. Only
        # this DVE op is on the serial chain; the fixup matmul runs QP
        # tiles later so the PE never stalls on the chain.
        if k < nt - 1:
            c_k = crow_pool.tile([1, Dk], carry_dt, tag="c")
            if k == 0:
                nc.vector.tensor_copy(c_k[:], ops[0:1, :])
            else:
                nc.vector.scalar_tensor_tensor(
                    out=c_k[:],
                    in0=carry[k - 1][:],
                    scalar=a_row[0:1, 0:1],
                    in1=ops[0:1, :],
                    op0=ALU.mult,
                    op1=ALU.add,
                )
            carry[k] = c_k

    def emit_back(k):
        # carry fixup + store for tile k
        ops = ops_tiles.pop(k)
        if k > 0:
            nc.tensor.matmul(
                ops[:],
                lhsT=a_rows.pop(k)[:],
                rhs=carry[k - 1][:],
                start=False,
                stop=True,
                skip_group_check=True,
            )
        # Stores go out in the raw (partition-major, rotated) layout:
        # out_raw[p, k, :] = y[k*T + (p-1) % T]. The host unpermutes. This
        # makes each batched SWDGE store 128 descriptors of STG*Dk*4 bytes.
        if k % STG == 0:
            osb_tiles[k // STG] = outsb_pool.tile(
                [T, STG * Dk], F32, tag="osb", name=f"osb_{k // STG}"
            )
        osb = osb_tiles[k // STG]
        dst = osb[:, (k % STG) * Dk : (k % STG + 1) * Dk]
        if k % 2 == 0:
            nc.scalar.copy(dst, ops[:])
        else:
            nc.vector.tensor_copy(dst, ops[:])
        if k % STG == STG - 1:
            g0 = k - (STG - 1)
            nc.gpsimd.dma_start(
                out=out_ap.rearrange("p (k d) -> p k d", d=Dk)[:, g0 : g0 + STG, :],
                in_=osb_tiles.pop(k // STG)[:].rearrange("p (k d) -> p k d", d=Dk),
            )

    for k in range(nt):
        emit_front(k)
        if k >= QP:
            emit_back(k - QP)
    for k in range(nt - QP, nt):
        emit_back(k)


def _host_constants():
    s = np.arange(T)[:, None]
    t = np.arange(T)[None, :]
    triu = (s <= t).astype(np.float32)  # incl upper
    # column-rotated mask: out partition t' holds position (t'-1) % T
    t_rot = (t - 1) % T
    mbias = np.where(s <= t_rot, 0.0, -3e38).astype(np.float32)
    ident = np.eye(T, dtype=np.float32)
    return triu, mbias, ident


def build_nc(Lk=L, Dk=DC, n_gather_chunks=8, mm_mode="f32r"):
    nt = Lk // T
    nc = bacc.Bacc(
        "TRN2", target_bir_lowering=False, debug=False, enable_asserts=False
    )
    hid_dt = BF16 if mm_mode == "bf16g" else F32
    hid = nc.dram_tensor("hid", [Lk, Dk], hid_dt, kind="ExternalInput").ap()
    p_t = nc.dram_tensor("p_t", [T, nt], F32, kind="ExternalInput").ap()
    m16 = nc.dram_tensor("m16", [16, Lk // 16], F32, kind="ExternalInput").ap()
    triu = nc.dram_tensor("triu", [T, T], F32, kind="ExternalInput").ap()
    mbias = nc.dram_tensor("mbias", [T, T], F32, kind="ExternalInput").ap()
    ident = nc.dram_tensor("ident", [T, T], F32, kind="ExternalInput").ap()
    # raw partition-major layout: out[p, k*Dk + d] = y[k*T + (p-1)%T, d]
    out = nc.dram_tensor("out", [T, (Lk // T) * Dk], F32, kind="ExternalOutput").ap()
    with tile.TileContext(nc) as tc:
        _dechunk_tile_kernel(
            tc, out, hid, p_t, m16, triu, mbias, ident, Lk, Dk, n_gather_chunks,
            mm_mode=mm_mode,
        )
    nc.compile()
    return nc


def unpermute_out(raw, Lk=L, Dk=DC):
    """raw (T, nt*Dk) partition-major rotated -> (Lk, Dk) sequence order."""
    nt = Lk // T
    raw = raw.reshape(T, nt, Dk)
    rr = raw[(np.arange(T) + 1) % T]  # rr[q, k] = y[k*T + q]
    return np.ascontiguousarray(rr.transpose(1, 0, 2).reshape(Lk, Dk))


def make_core_inputs(hid_c, p_c, m_c, Lk=L, mm_mode="f32r"):
    """Per-core input map. hid_c (Lk, Dk) f32; p_c, m_c (Lk,) f32."""
    nt = Lk // T
    triu, mbias, ident = _host_constants()
    if mm_mode == "bf16g":
        import ml_dtypes

        hid_arr = np.ascontiguousarray(np.asarray(hid_c).astype(ml_dtypes.bfloat16))
    else:
        hid_arr = np.ascontiguousarray(hid_c, dtype=np.float32)
    return {
        "hid": hid_arr,
        "p_t": np.ascontiguousarray(
            p_c.astype(np.float32).reshape(nt, T).T
        ),  # tile-major (T, nt)
        "m16": np.ascontiguousarray(m_c.astype(np.float32).reshape(Lk // 16, 16).T),
        "triu": triu,
        "mbias": mbias,
        "ident": ident,
    }


_NC_CACHE = {}
MM_MODE = "f32r"
N_GATHER_CHUNKS = 8


def _get_nc():
    key = (L, DC, N_GATHER_CHUNKS, MM_MODE)
    if key not in _NC_CACHE:
        _NC_CACHE[key] = build_nc(L, DC, N_GATHER_CHUNKS, MM_MODE)
    return _NC_CACHE[key]


def run_cores(hidden_states, boundary_mask, boundary_prob, trace=False, **kw):
    """Shard, run on 8 NeuronCores, reassemble. Returns (out, BassKernelResults)."""
    from concourse.bass_utils import run_bass_kernel_spmd

    hidden_states = np.asarray(hidden_states, dtype=np.float32)
    boundary_mask = np.asarray(boundary_mask)
    boundary_prob = np.asarray(boundary_prob, dtype=np.float32)
    assert hidden_states.shape == (B, L, D)

    nc = _get_nc()
    in_maps = []
    for c in range(N_CORES):
        b, dh = c // 2, c % 2
        in_maps.append(
            make_core_inputs(
                hidden_states[b, :, dh * DC : (dh + 1) * DC],
                boundary_prob[b, :, 1],
                boundary_mask[b].astype(np.float32),
                mm_mode=MM_MODE,
            )
        )
    res = run_bass_kernel_spmd(nc, in_maps, list(range(N_CORES)), trace=trace, **kw)
    out = np.empty((B, L, D), dtype=np.float32)
    for c in range(N_CORES):
        b, dh = c // 2, c % 2
        out[b, :, dh * DC : (dh + 1) * DC] = unpermute_out(res.results[c]["out"])
    return out, res


def kernel(hidden_states, boundary_mask, boundary_prob):
    out, _ = run_cores(hidden_states, boundary_mask, boundary_prob, trace=False)
    return out


# revision 47
# speedup vs baseline: 1.2020x; 1.2020x over previous
"""DeChunk EMA-scan kernel for Trainium2 (Bass/Tile), 8 NeuronCores.

Problem: out[b,t,:] = p_t * x_t + (1-p_t) * out[b,t-1,:], where
x_t = hidden[b, idx_t, :], idx = cumsum(boundary_mask)-1,
p = clip(boundary_prob[...,1], EPS, 1-EPS) with p[:,0]=1.

Sharding: pure data parallel. core c handles batch b=c//2 and channel half
dh=c%2 (512 of 1024 channels). No cross-core communication.

Device algorithm per core (L=8192, Dc=512):
  - idx = global cumsum(mask)-1 via triangular matmuls (tile-local cumsum +
    tile-offset fixup), converted to the int16 "16-partition wrapped" layout
    dma_gather wants.
  - gathered = hid[idx] via dma_gather in 8 chunks, landing tile-major:
    position t -> partition t%128, free block t//128.
  - Blocked scan over 64 tiles of T=128: out_tile = Mk @ (p*x) + A (x) carry,
    where Mk[t,s] = prod_{r=s+1..t} a_r * p_s = exp(S_t - S_s + log p_s)
    (tile-local S = cumsum log a), built with one K=1 matmul (broadcast S_t
    down partitions), a mask add (-3e38 upper part), and one ScalarE exp
    with per-partition bias (log p_s - S_s). A_t = exp(S_t).
  - Carry c_k = out_k[127,:] feeds tile k+1 via a K=1 bf16 matmul accumulated
    into the same PSUM bank as the big matmul.
"""

import sys

for _p in ("/opt/trn_rl_repo", "/root/.axon_site/_ro/trn_rl_repo"):
    if _p not in sys.path:
        sys.path.insert(0, _p)

import numpy as np
from contextlib import ExitStack

import concourse.bass as bass
import concourse.tile as tile
from concourse import bacc, mybir
from concourse._compat import with_exitstack

B, L, D = 4, 8192, 1024
N_CORES = 8
DC = D // 2  # channels per core
T = 128  # scan tile length
EPS = 1e-4
F32 = mybir.dt.float32
BF16 = mybir.dt.bfloat16
I16 = mybir.dt.int16
ALU = mybir.AluOpType
ACTF = mybir.ActivationFunctionType


@with_exitstack
def _dechunk_tile_kernel(
    ctx: ExitStack,
    tc: "tile.TileContext",
    out_ap: bass.AP,
    hid_ap: bass.AP,
    p_ap: bass.AP,
    m_ap: bass.AP,
    triu_ap: bass.AP,
    mbias_ap: bass.AP,
    ident_ap: bass.AP,
    Lk: int,
    Dk: int,
    n_gather_chunks: int,
    mm_mode: str = "f32r",
):
    nc = tc.nc
    F32R = mybir.dt.float32r
    # mm_mode: "f32r" = float32r big/fixup matmuls (1 cycle/row, ~17-bit
    # mantissa products); "f32" = plain fp32 matmuls (4 cycles/row, safest);
    # "bf16" = fp32 big matmul + bf16 carry/fixup (~2e-3 absmax err).
    assert mm_mode in ("f32r", "f32", "bf16", "bf16g")
    # "bf16g": hidden states shipped/gathered as bf16, M-matrix bf16 — big
    # matmul at true 1 cycle/row and half the gather bytes (~4e-3 absmax).
    mm_dt = {"f32r": F32R, "bf16g": BF16}.get(mm_mode, F32)
    # carry path must avoid float32r (DVE has no f32r encodings); bf16 keeps
    # the fixup matmul at 1 cycle/row and costs ~2e-3 absmax error.
    carry_dt = F32 if mm_mode == "f32" else BF16
    nt = Lk // T  # number of scan tiles
    assert Lk % (16 * n_gather_chunks) == 0
    ch_idx = Lk // n_gather_chunks  # indices per gather chunk
    assert ch_idx % T == 0
    ch_tiles = ch_idx // T  # scan tiles per gather chunk

    STG = 4  # output tiles batched per store DMA
    assert nt % STG == 0

    const = ctx.enter_context(tc.tile_pool(name="const", bufs=1))
    sb = ctx.enter_context(tc.tile_pool(name="sb", bufs=1))
    gat_pool = ctx.enter_context(tc.tile_pool(name="gat", bufs=n_gather_chunks))
    lhs_pool = ctx.enter_context(tc.tile_pool(name="lhs", bufs=4))
    stage_pool = ctx.enter_context(tc.tile_pool(name="stage", bufs=4))
    outsb_pool = ctx.enter_context(tc.tile_pool(name="outsb", bufs=2))
    crow_pool = ctx.enter_context(tc.tile_pool(name="crow", bufs=6))
    psum_small = ctx.enter_context(tc.tile_pool(name="psum_sm", bufs=2, space="PSUM"))
    psum_scol = ctx.enter_context(tc.tile_pool(name="psum_scol", bufs=2, space="PSUM"))
    psum_out = ctx.enter_context(tc.tile_pool(name="psum_out", bufs=4, space="PSUM"))

    # ---- load constants / small inputs
    ctri = const.tile([T, T], F32)
    nc.sync.dma_start(out=ctri[:], in_=triu_ap)
    cmb = const.tile([T, T], F32)
    nc.sync.dma_start(out=cmb[:], in_=mbias_ap)
    cid = const.tile([T, T], F32)
    nc.sync.dma_start(out=cid[:], in_=ident_ap)
    p_sb = const.tile([T, nt], F32)
    nc.sync.dma_start(out=p_sb[:], in_=p_ap)
    m16_sb = const.tile([16, Lk // 16], F32)
    nc.sync.dma_start(out=m16_sb[:], in_=m_ap)

    # ---- p processing: clip, p0=1, a=1-p (a0=tiny), logs
    pc = sb.tile([T, nt], F32, tag="pc")
    nc.vector.tensor_scalar(
        out=pc[:], in0=p_sb[:], scalar1=EPS, scalar2=1.0 - EPS, op0=ALU.max, op1=ALU.min
    )
    nc.vector.memset(pc[0:1, 0:1], 1.0)
    av = sb.tile([T, nt], F32, tag="av")
    nc.vector.tensor_scalar(
        out=av[:], in0=pc[:], scalar1=-1.0, scalar2=1.0, op0=ALU.mult, op1=ALU.add
    )
    nc.vector.memset(av[0:1, 0:1], 1e-30)
    loga = sb.tile([T, nt], F32, tag="loga")
    nc.scalar.activation(loga[:], av[:], ACTF.Ln)
    logp = sb.tile([T, nt], F32, tag="logp")
    nc.scalar.activation(logp[:], pc[:], ACTF.Ln)

    # ---- S = tile-local inclusive cumsum of log a (one matmul for all tiles)
    S_ps = psum_small.tile([T, nt], F32, tag="small")
    nc.tensor.matmul(S_ps[:], lhsT=ctri[:], rhs=loga[:], start=True, stop=True)
    S_sb = sb.tile([T, nt], F32, tag="S_sb")
    nc.vector.tensor_copy(S_sb[:], S_ps[:])
    biasv = sb.tile([T, nt], F32, tag="biasv")
    nc.vector.tensor_tensor(out=biasv[:], in0=logp[:], in1=S_sb[:], op=ALU.subtract)
    # S_T[k, t] = S_sb[t, k], rotated (column t' holds position (t'-1)%T) and
    # flattened to a single-partition row S_flat[0, k*T + t'] so per-tile S
    # rows are free-dim slices (engine APs may only start at partitions
    # 0/32/64/96).
    S_T_ps = psum_small.tile([nt, T], F32, tag="small")
    nc.tensor.transpose(S_T_ps[:], S_sb[:], cid[:])
    S_T_rot = sb.tile([nt, T], F32, tag="S_T_rot")
    nc.vector.tensor_copy(S_T_rot[:, 1:T], S_T_ps[:, 0 : T - 1])
    nc.vector.tensor_copy(S_T_rot[:, 0:1], S_T_ps[:, T - 1 : T])
    S_flat = sb.tile([1, nt * T], F32, tag="S_flat")
    nc.sync.dma_start(
        out=S_flat[:].rearrange("p (k t) -> p k t", t=T), in_=S_T_rot[:]
    )

    # ---- idx = global cumsum(mask) - 1, directly in the int16
    # "16-partition wrapped" layout dma_gather wants: idx16[q, s] = idx[s*16+q].
    ns = Lk // 16
    zeros16 = sb.tile([16, ns], F32, tag="zeros16")
    nc.vector.memset(zeros16[:], 0.0)
    # csw[q, s] = sum_{s'<=s} m16[q, s']  (per-partition running sum)
    csw = sb.tile([16, ns], F32, tag="csw")
    nc.vector.tensor_tensor_scan(
        out=csw[:], data0=m16_sb[:], data1=zeros16[:], initial=0.0,
        op0=ALU.add, op1=ALU.add,
    )
    # totcum[s] = total mask count through column s (sum over the 16 rows)
    totcum_ps = psum_small.tile([1, ns], F32, tag="small")
    nc.tensor.matmul(
        totcum_ps[:], lhsT=ctri[0:16, T - 1 : T], rhs=csw[:], start=True, stop=True
    )
    # offs16[s] = totcum[s-1], offs16[0] = 0
    offs16 = sb.tile([1, ns], F32, tag="offs16")
    nc.vector.memset(offs16[0:1, 0:1], 0.0)
    nc.vector.tensor_copy(offs16[0:1, 1:ns], totcum_ps[0:1, 0 : ns - 1])
    # idx16_ps[q, s] = (within-column cumsum)[q, s] + offs16[s]
    idx16_ps = psum_small.tile([16, ns], F32, tag="small")
    nc.tensor.matmul(
        idx16_ps[:], lhsT=ctri[0:16, 0:16], rhs=m16_sb[:], start=True, stop=False
    )
    nc.tensor.matmul(
        idx16_ps[:], lhsT=ctri[0:1, 0:16], rhs=offs16[:], start=False, stop=True
    )
    idx16 = sb.tile([T, ns], I16, tag="idx16")
    nc.vector.tensor_scalar(
        out=idx16[0:16, :], in0=idx16_ps[:], scalar1=-1.0, scalar2=None, op0=ALU.add
    )
    # replicate to all 8 partition groups (one per Q7 core) via SBUF->SBUF DMA
    nc.sync.dma_start(out=idx16[16:32, :], in_=idx16[0:16, :])
    nc.sync.dma_start(out=idx16[32:64, :], in_=idx16[0:32, :])
    nc.sync.dma_start(out=idx16[64:128, :], in_=idx16[0:64, :])

    # ---- gather: hid[idx] in chunks, tile-major layout. Emission is ~8.4ns
    # per 2KiB descriptor on the Q7, so gathers are emitted just-in-time,
    # interleaved with the main loop (a monolithic up-front emission would
    # serialize ~70us ahead of the first store, which shares the Q7).
    gat_tiles = {}

    def emit_gather(c):
        if c >= n_gather_chunks:
            return
        g_t = gat_pool.tile(
            [T, ch_tiles * Dk], mm_dt, tag="gat", name=f"gat_{c}"
        )
        g3 = g_t[:].rearrange("p (j d) -> p j d", d=Dk)
        nc.gpsimd.dma_gather(
            out_ap=g3,
            in_ap=hid_ap.bitcast(mm_dt),
            idxs_ap=idx16[:, c * (ch_idx // 16) : (c + 1) * (ch_idx // 16)],
            num_idxs=ch_idx,
            num_idxs_reg=ch_idx,
            elem_size=Dk,
            queue_num=1 if nc.num_swdge_queues > 1 else 0,
        )
        gat_tiles[c] = g_t

    GA = 2  # chunks of gather-ahead
    for c in range(GA):
        emit_gather(c)

    # ---- main blocked scan
    # Output rows are ROTATED by one: out partition t' holds position
    # (t'-1) % T, so the carry row (position T-1) lands at partition 0,
    # which is the only legal engine-read start. The host unpermutes.
    SCH = 4  # tiles per Scol-broadcast chunk (N = SCH*T = 512 per matmul)
    QP = 2  # software pipeline depth: bigmm runs QP tiles ahead of fixup
    assert nt % SCH == 0
    carry = {}  # k -> carry tile holding c_k
    a_rows = {}
    ops_tiles = {}
    osb_tiles = {}

    def emit_front(k):
        # lhsT build + big matmul for tile k (independent of the carry chain)
        if k % SCH == 0:
            tmp_ps = psum_scol.tile([T, SCH * T], F32, tag="scol")
            nc.tensor.matmul(
                tmp_ps[:],
                lhsT=ctri[0:1, :],
                rhs=S_flat[0:1, k * T : (k + SCH) * T],
                start=True,
                stop=True,
            )
            emit_front.tmp_ps = tmp_ps
        tmp_ps = emit_front.tmp_ps
        j = k % SCH
        nc.vector.tensor_tensor(
            out=tmp_ps[:, j * T : (j + 1) * T],
            in0=tmp_ps[:, j * T : (j + 1) * T],
            in1=cmb[:],
            op=ALU.add,
        )
        lhsT_k = lhs_pool.tile([T, T], mm_dt, tag="lhsT")
        nc.scalar.activation(
            lhsT_k[:],
            tmp_ps[:, j * T : (j + 1) * T],
            ACTF.Exp,
            bias=biasv[:, k : k + 1],
            scale=1.0,
        )
        if k > 0:
            a_row = stage_pool.tile([1, T], carry_dt, tag="a_row")
            nc.scalar.activation(
                a_row[:], S_flat[0:1, k * T : (k + 1) * T], ACTF.Exp
            )
            a_rows[k] = a_row
        ops = psum_out.tile([T, Dk], F32, tag="ops")
        rhs = gat_tiles[k // ch_tiles][:].rearrange("p (j d) -> p j d", d=Dk)[
            :, k % ch_tiles, :
        ]
        nc.tensor.matmul(ops[:], lhsT=lhsT_k[:], rhs=rhs, start=True, stop=True)
        ops_tiles[k] = ops

        # Fast serial carry: c_k = A_k[T-1] * c_{k-1} + ops0[row 0]. Only
        # this DVE op is on the serial chain; the fixup matmul runs QP
        # tiles later so the PE never stalls on the chain.
        if k < nt - 1:
            c_k = crow_pool.tile([1, Dk], carry_dt, tag="c")
            if k == 0:
                nc.vector.tensor_copy(c_k[:], ops[0:1, :])
            else:
                nc.vector.scalar_tensor_tensor(
                    out=c_k[:],
                    in0=carry[k - 1][:],
                    scalar=a_row[0:1, 0:1],
                    in1=ops[0:1, :],
                    op0=ALU.mult,
                    op1=ALU.add,
                )
            carry[k] = c_k

    def emit_back(k):
        # carry fixup + store for tile k
        ops = ops_tiles.pop(k)
        if k > 0:
            nc.tensor.matmul(
                ops[:],
                lhsT=a_rows.pop(k)[:],
                rhs=carry[k - 1][:],
                start=False,
                stop=True,
                skip_group_check=True,
            )
        # Stores go out in the raw (partition-major, rotated) layout:
        # out_raw[p, k, :] = y[k*T + (p-1) % T]. The host unpermutes. This
        # makes each batched SWDGE store 128 descriptors of STG*Dk*4 bytes.
        if k % STG == 0:
            osb_tiles[k // STG] = outsb_pool.tile(
                [T, STG * Dk], F32, tag="osb", name=f"osb_{k // STG}"
            )
        osb = osb_tiles[k // STG]
        dst = osb[:, (k % STG) * Dk : (k % STG + 1) * Dk]
        if k % 2 == 0:
            nc.scalar.copy(dst, ops[:])
        else:
            nc.vector.tensor_copy(dst, ops[:])
        if k % STG == STG - 1:
            g0 = k - (STG - 1)
            nc.gpsimd.dma_start(
                out=out_ap.rearrange("p (k d) -> p k d", d=Dk)[:, g0 : g0 + STG, :],
                in_=osb_tiles.pop(k // STG)[:].rearrange("p (k d) -> p k d", d=Dk),
            )

    for k in range(nt):
        if k % ch_tiles == 0:
            emit_gather(k // ch_tiles + GA)
        emit_front(k)
        if k >= QP:
            emit_back(k - QP)
    for k in range(nt - QP, nt):
        emit_back(k)


def _host_constants():
    s = np.arange(T)[:, None]
    t = np.arange(T)[None, :]
    triu = (s <= t).astype(np.float32)  # incl upper
    # column-rotated mask: out partition t' holds position (t'-1) % T
    t_rot = (t - 1) % T
    mbias = np.where(s <= t_rot, 0.0, -3e38).astype(np.float32)
    ident = np.eye(T, dtype=np.float32)
    return triu, mbias, ident


def build_nc(Lk=L, Dk=DC, n_gather_chunks=16, mm_mode="f32r"):
    nt = Lk // T
    nc = bacc.Bacc(
        "TRN2",
        target_bir_lowering=False,
        debug=False,
        enable_asserts=False,
        num_swdge_queues=2,
    )
    hid_dt = BF16 if mm_mode == "bf16g" else F32
    hid = nc.dram_tensor("hid", [Lk, Dk], hid_dt, kind="ExternalInput").ap()
    p_t = nc.dram_tensor("p_t", [T, nt], F32, kind="ExternalInput").ap()
    m16 = nc.dram_tensor("m16", [16, Lk // 16], F32, kind="ExternalInput").ap()
    triu = nc.dram_tensor("triu", [T, T], F32, kind="ExternalInput").ap()
    mbias = nc.dram_tensor("mbias", [T, T], F32, kind="ExternalInput").ap()
    ident = nc.dram_tensor("ident", [T, T], F32, kind="ExternalInput").ap()
    # raw partition-major layout: out[p, k*Dk + d] = y[k*T + (p-1)%T, d]
    out = nc.dram_tensor("out", [T, (Lk // T) * Dk], F32, kind="ExternalOutput").ap()
    with tile.TileContext(nc) as tc:
        _dechunk_tile_kernel(
            tc, out, hid, p_t, m16, triu, mbias, ident, Lk, Dk, n_gather_chunks,
            mm_mode=mm_mode,
        )
    nc.compile()
    return nc


def unpermute_out(raw, Lk=L, Dk=DC):
    """raw (T, nt*Dk) partition-major rotated -> (Lk, Dk) sequence order."""
    nt = Lk // T
    raw = raw.reshape(T, nt, Dk)
    rr = raw[(np.arange(T) + 1) % T]  # rr[q, k] = y[k*T + q]
    return np.ascontiguousarray(rr.transpose(1, 0, 2).reshape(Lk, Dk))


def make_core_inputs(hid_c, p_c, m_c, Lk=L, mm_mode="f32r"):
    """Per-core input map. hid_c (Lk, Dk) f32; p_c, m_c (Lk,) f32."""
    nt = Lk // T
    triu, mbias, ident = _host_constants()
    if mm_mode == "bf16g":
        import ml_dtypes

        hid_arr = np.ascontiguousarray(np.asarray(hid_c).astype(ml_dtypes.bfloat16))
    else:
        hid_arr = np.ascontiguousarray(hid_c, dtype=np.float32)
    return {
        "hid": hid_arr,
        "p_t": np.ascontiguousarray(
            p_c.astype(np.float32).reshape(nt, T).T
        ),  # tile-major (T, nt)
        "m16": np.ascontiguousarray(m_c.astype(np.float32).reshape(Lk // 16, 16).T),
        "triu": triu,
        "mbias": mbias,
        "ident": ident,
    }


_NC_CACHE = {}
MM_MODE = "f32r"
N_GATHER_CHUNKS = 16


def _get_nc():
    key = (L, DC, N_GATHER_CHUNKS, MM_MODE)
    if key not in _NC_CACHE:
        _NC_CACHE[key] = build_nc(L, DC, N_GATHER_CHUNKS, MM_MODE)
    return _NC_CACHE[key]


def run_cores(hidden_states, boundary_mask, boundary_prob, trace=False, **kw):
    """Shard, run on 8 NeuronCores, reassemble. Returns (out, BassKernelResults)."""
    from concourse.bass_utils import run_bass_kernel_spmd

    hidden_states = np.asarray(hidden_states, dtype=np.float32)
    boundary_mask = np.asarray(boundary_mask)
    boundary_prob = np.asarray(boundary_prob, dtype=np.float32)
    assert hidden_states.shape == (B, L, D)

    nc = _get_nc()
    in_maps = []
    for c in range(N_CORES):
        b, dh = c // 2, c % 2
        in_maps.append(
            make_core_inputs(
                hidden_states[b, :, dh * DC : (dh + 1) * DC],
                boundary_prob[b, :, 1],
                boundary_mask[b].astype(np.float32),
                mm_mode=MM_MODE,
            )
        )
    res = run_bass_kernel_spmd(nc, in_maps, list(range(N_CORES)), trace=trace, **kw)
    out = np.empty((B, L, D), dtype=np.float32)
    for c in range(N_CORES):
        b, dh = c // 2, c % 2
        out[b, :, dh * DC : (dh + 1) * DC] = unpermute_out(res.results[c]["out"])
    return out, res


def kernel(hidden_states, boundary_mask, boundary_prob):
    out, _ = run_cores(hidden_states, boundary_mask, boundary_prob, trace=False)
    return out


# revision 48
# speedup vs baseline: 1.2283x; 1.0219x over previous
"""DeChunk EMA-scan kernel for Trainium2 (Bass/Tile), 8 NeuronCores.

Problem: out[b,t,:] = p_t * x_t + (1-p_t) * out[b,t-1,:], where
x_t = hidden[b, idx_t, :], idx = cumsum(boundary_mask)-1,
p = clip(boundary_prob[...,1], EPS, 1-EPS) with p[:,0]=1.

Sharding: pure data parallel. core c handles batch b=c//2 and channel half
dh=c%2 (512 of 1024 channels). No cross-core communication.

Device algorithm per core (L=8192, Dc=512):
  - idx = global cumsum(mask)-1 via triangular matmuls (tile-local cumsum +
    tile-offset fixup), converted to the int16 "16-partition wrapped" layout
    dma_gather wants.
  - gathered = hid[idx] via dma_gather in 8 chunks, landing tile-major:
    position t -> partition t%128, free block t//128.
  - Blocked scan over 64 tiles of T=128: out_tile = Mk @ (p*x) + A (x) carry,
    where Mk[t,s] = prod_{r=s+1..t} a_r * p_s = exp(S_t - S_s + log p_s)
    (tile-local S = cumsum log a), built with one K=1 matmul (broadcast S_t
    down partitions), a mask add (-3e38 upper part), and one ScalarE exp
    with per-partition bias (log p_s - S_s). A_t = exp(S_t).
  - Carry c_k = out_k[127,:] feeds tile k+1 via a K=1 bf16 matmul accumulated
    into the same PSUM bank as the big matmul.
"""

import sys

for _p in ("/opt/trn_rl_repo", "/root/.axon_site/_ro/trn_rl_repo"):
    if _p not in sys.path:
        sys.path.insert(0, _p)

import numpy as np
from contextlib import ExitStack

import concourse.bass as bass
import concourse.tile as tile
from concourse import bacc, mybir
from concourse._compat import with_exitstack

B, L, D = 4, 8192, 1024
N_CORES = 8
DC = D // 2  # channels per core
T = 128  # scan tile length
EPS = 1e-4
F32 = mybir.dt.float32
BF16 = mybir.dt.bfloat16
I16 = mybir.dt.int16
ALU = mybir.AluOpType
ACTF = mybir.ActivationFunctionType


@with_exitstack
def _dechunk_tile_kernel(
    ctx: ExitStack,
    tc: "tile.TileContext",
    out_ap: bass.AP,
    hid_ap: bass.AP,
    p_ap: bass.AP,
    m_ap: bass.AP,
    triu_ap: bass.AP,
    mbias_ap: bass.AP,
    ident_ap: bass.AP,
    Lk: int,
    Dk: int,
    n_gather_chunks: int,
    mm_mode: str = "f32r",
):
    nc = tc.nc
    F32R = mybir.dt.float32r
    # mm_mode: "f32r" = float32r big/fixup matmuls (1 cycle/row, ~17-bit
    # mantissa products); "f32" = plain fp32 matmuls (4 cycles/row, safest);
    # "bf16" = fp32 big matmul + bf16 carry/fixup (~2e-3 absmax err).
    assert mm_mode in ("f32r", "f32", "bf16", "bf16g")
    # "bf16g": hidden states shipped/gathered as bf16, M-matrix bf16 — big
    # matmul at true 1 cycle/row and half the gather bytes (~4e-3 absmax).
    mm_dt = {"f32r": F32R, "bf16g": BF16}.get(mm_mode, F32)
    # carry path must avoid float32r (DVE has no f32r encodings); bf16 keeps
    # the fixup matmul at 1 cycle/row and costs ~2e-3 absmax error.
    carry_dt = F32 if mm_mode == "f32" else BF16
    nt = Lk // T  # number of scan tiles
    assert Lk % (16 * n_gather_chunks) == 0
    ch_idx = Lk // n_gather_chunks  # indices per gather chunk
    assert ch_idx % T == 0
    ch_tiles = ch_idx // T  # scan tiles per gather chunk

    STG = 4  # output tiles batched per store DMA
    assert nt % STG == 0

    const = ctx.enter_context(tc.tile_pool(name="const", bufs=1))
    sb = ctx.enter_context(tc.tile_pool(name="sb", bufs=1))
    gat_pool = ctx.enter_context(tc.tile_pool(name="gat", bufs=n_gather_chunks))
    lhs_pool = ctx.enter_context(tc.tile_pool(name="lhs", bufs=4))
    stage_pool = ctx.enter_context(tc.tile_pool(name="stage", bufs=4))
    outsb_pool = ctx.enter_context(tc.tile_pool(name="outsb", bufs=2))
    crow_pool = ctx.enter_context(tc.tile_pool(name="crow", bufs=6))
    psum_small = ctx.enter_context(tc.tile_pool(name="psum_sm", bufs=2, space="PSUM"))
    psum_scol = ctx.enter_context(tc.tile_pool(name="psum_scol", bufs=2, space="PSUM"))
    psum_out = ctx.enter_context(tc.tile_pool(name="psum_out", bufs=4, space="PSUM"))

    # ---- load constants / small inputs
    ctri = const.tile([T, T], F32)
    nc.sync.dma_start(out=ctri[:], in_=triu_ap)
    cmb = const.tile([T, T], F32)
    nc.sync.dma_start(out=cmb[:], in_=mbias_ap)
    cid = const.tile([T, T], F32)
    nc.sync.dma_start(out=cid[:], in_=ident_ap)
    p_sb = const.tile([T, nt], F32)
    nc.sync.dma_start(out=p_sb[:], in_=p_ap)
    m16_sb = const.tile([16, Lk // 16], F32)
    nc.sync.dma_start(out=m16_sb[:], in_=m_ap)

    # ---- p processing: clip, p0=1, a=1-p (a0=tiny), logs
    pc = sb.tile([T, nt], F32, tag="pc")
    nc.vector.tensor_scalar(
        out=pc[:], in0=p_sb[:], scalar1=EPS, scalar2=1.0 - EPS, op0=ALU.max, op1=ALU.min
    )
    nc.vector.memset(pc[0:1, 0:1], 1.0)
    av = sb.tile([T, nt], F32, tag="av")
    nc.vector.tensor_scalar(
        out=av[:], in0=pc[:], scalar1=-1.0, scalar2=1.0, op0=ALU.mult, op1=ALU.add
    )
    nc.vector.memset(av[0:1, 0:1], 1e-30)
    loga = sb.tile([T, nt], F32, tag="loga")
    nc.scalar.activation(loga[:], av[:], ACTF.Ln)
    logp = sb.tile([T, nt], F32, tag="logp")
    nc.scalar.activation(logp[:], pc[:], ACTF.Ln)

    # ---- S = tile-local inclusive cumsum of log a (one matmul for all tiles)
    S_ps = psum_small.tile([T, nt], F32, tag="small")
    nc.tensor.matmul(S_ps[:], lhsT=ctri[:], rhs=loga[:], start=True, stop=True)
    S_sb = sb.tile([T, nt], F32, tag="S_sb")
    nc.vector.tensor_copy(S_sb[:], S_ps[:])
    biasv = sb.tile([T, nt], F32, tag="biasv")
    nc.vector.tensor_tensor(out=biasv[:], in0=logp[:], in1=S_sb[:], op=ALU.subtract)
    # S_T[k, t] = S_sb[t, k], rotated (column t' holds position (t'-1)%T) and
    # flattened to a single-partition row S_flat[0, k*T + t'] so per-tile S
    # rows are free-dim slices (engine APs may only start at partitions
    # 0/32/64/96).
    S_T_ps = psum_small.tile([nt, T], F32, tag="small")
    nc.tensor.transpose(S_T_ps[:], S_sb[:], cid[:])
    S_T_rot = sb.tile([nt, T], F32, tag="S_T_rot")
    nc.vector.tensor_copy(S_T_rot[:, 1:T], S_T_ps[:, 0 : T - 1])
    nc.vector.tensor_copy(S_T_rot[:, 0:1], S_T_ps[:, T - 1 : T])
    S_flat = sb.tile([1, nt * T], F32, tag="S_flat")
    nc.sync.dma_start(
        out=S_flat[:].rearrange("p (k t) -> p k t", t=T), in_=S_T_rot[:]
    )

    # ---- idx = global cumsum(mask) - 1, directly in the int16
    # "16-partition wrapped" layout dma_gather wants: idx16[q, s] = idx[s*16+q].
    ns = Lk // 16
    zeros16 = sb.tile([16, ns], F32, tag="zeros16")
    nc.vector.memset(zeros16[:], 0.0)
    # csw[q, s] = sum_{s'<=s} m16[q, s']  (per-partition running sum)
    csw = sb.tile([16, ns], F32, tag="csw")
    nc.vector.tensor_tensor_scan(
        out=csw[:], data0=m16_sb[:], data1=zeros16[:], initial=0.0,
        op0=ALU.add, op1=ALU.add,
    )
    # totcum[s] = total mask count through column s (sum over the 16 rows)
    totcum_ps = psum_small.tile([1, ns], F32, tag="small")
    nc.tensor.matmul(
        totcum_ps[:], lhsT=ctri[0:16, T - 1 : T], rhs=csw[:], start=True, stop=True
    )
    # offs16[s] = totcum[s-1], offs16[0] = 0
    offs16 = sb.tile([1, ns], F32, tag="offs16")
    nc.vector.memset(offs16[0:1, 0:1], 0.0)
    nc.vector.tensor_copy(offs16[0:1, 1:ns], totcum_ps[0:1, 0 : ns - 1])
    # idx16_ps[q, s] = (within-column cumsum)[q, s] + offs16[s]
    idx16_ps = psum_small.tile([16, ns], F32, tag="small")
    nc.tensor.matmul(
        idx16_ps[:], lhsT=ctri[0:16, 0:16], rhs=m16_sb[:], start=True, stop=False
    )
    nc.tensor.matmul(
        idx16_ps[:], lhsT=ctri[0:1, 0:16], rhs=offs16[:], start=False, stop=True
    )
    idx16 = sb.tile([T, ns], I16, tag="idx16")
    nc.vector.tensor_scalar(
        out=idx16[0:16, :], in0=idx16_ps[:], scalar1=-1.0, scalar2=None, op0=ALU.add
    )
    # replicate to all 8 partition groups (one per Q7 core) via SBUF->SBUF DMA
    nc.sync.dma_start(out=idx16[16:32, :], in_=idx16[0:16, :])
    nc.sync.dma_start(out=idx16[32:64, :], in_=idx16[0:32, :])
    nc.sync.dma_start(out=idx16[64:128, :], in_=idx16[0:64, :])

    # ---- gather: hid[idx] in chunks, tile-major layout. Emission is ~8.4ns
    # per 2KiB descriptor on the Q7, so gathers are emitted just-in-time,
    # interleaved with the main loop (a monolithic up-front emission would
    # serialize ~70us ahead of the first store, which shares the Q7).
    gat_tiles = {}

    def emit_gather(c):
        if c >= n_gather_chunks:
            return
        g_t = gat_pool.tile(
            [T, ch_tiles * Dk], mm_dt, tag="gat", name=f"gat_{c}"
        )
        g3 = g_t[:].rearrange("p (j d) -> p j d", d=Dk)
        nc.gpsimd.dma_gather(
            out_ap=g3,
            in_ap=hid_ap.bitcast(mm_dt),
            idxs_ap=idx16[:, c * (ch_idx // 16) : (c + 1) * (ch_idx // 16)],
            num_idxs=ch_idx,
            num_idxs_reg=ch_idx,
            elem_size=Dk,
            queue_num=1 if nc.num_swdge_queues > 1 else 0,
        )
        gat_tiles[c] = g_t

    GA = 4  # chunks of gather-ahead
    for c in range(GA):
        emit_gather(c)

    # ---- main blocked scan
    # Output rows are ROTATED by one: out partition t' holds position
    # (t'-1) % T, so the carry row (position T-1) lands at partition 0,
    # which is the only legal engine-read start. The host unpermutes.
    SCH = 4  # tiles per Scol-broadcast chunk (N = SCH*T = 512 per matmul)
    QP = 2  # software pipeline depth: bigmm runs QP tiles ahead of fixup
    assert nt % SCH == 0
    carry = {}  # k -> carry tile holding c_k
    a_rows = {}
    ops_tiles = {}
    osb_tiles = {}

    def emit_front(k):
        # lhsT build + big matmul for tile k (independent of the carry chain)
        if k % SCH == 0:
            tmp_ps = psum_scol.tile([T, SCH * T], F32, tag="scol")
            nc.tensor.matmul(
                tmp_ps[:],
                lhsT=ctri[0:1, :],
                rhs=S_flat[0:1, k * T : (k + SCH) * T],
                start=True,
                stop=True,
            )
            emit_front.tmp_ps = tmp_ps
        tmp_ps = emit_front.tmp_ps
        j = k % SCH
        nc.vector.tensor_tensor(
            out=tmp_ps[:, j * T : (j + 1) * T],
            in0=tmp_ps[:, j * T : (j + 1) * T],
            in1=cmb[:],
            op=ALU.add,
        )
        lhsT_k = lhs_pool.tile([T, T], mm_dt, tag="lhsT")
        nc.scalar.activation(
            lhsT_k[:],
            tmp_ps[:, j * T : (j + 1) * T],
            ACTF.Exp,
            bias=biasv[:, k : k + 1],
            scale=1.0,
        )
        if k > 0:
            a_row = stage_pool.tile([1, T], carry_dt, tag="a_row")
            nc.scalar.activation(
                a_row[:], S_flat[0:1, k * T : (k + 1) * T], ACTF.Exp
            )
            a_rows[k] = a_row
        ops = psum_out.tile([T, Dk], F32, tag="ops")
        rhs = gat_tiles[k // ch_tiles][:].rearrange("p (j d) -> p j d", d=Dk)[
            :, k % ch_tiles, :
        ]
        nc.tensor.matmul(ops[:], lhsT=lhsT_k[:], rhs=rhs, start=True, stop=True)
        ops_tiles[k] = ops

        # Fast serial carry: c_k = A_k[T-1] * c_{k-1} + ops0[row 0]. Only
        # this DVE op is on the serial chain; the fixup matmul runs QP
        # tiles later so the PE never stalls on the chain.
        if k < nt - 1:
            c_k = crow_pool.tile([1, Dk], carry_dt, tag="c")
            if k == 0:
                nc.vector.tensor_copy(c_k[:], ops[0:1, :])
            else:
                nc.vector.scalar_tensor_tensor(
                    out=c_k[:],
                    in0=carry[k - 1][:],
                    scalar=a_row[0:1, 0:1],
                    in1=ops[0:1, :],
                    op0=ALU.mult,
                    op1=ALU.add,
                )
            carry[k] = c_k

    def emit_back(k):
        # carry fixup + store for tile k
        ops = ops_tiles.pop(k)
        if k > 0:
            nc.tensor.matmul(
                ops[:],
                lhsT=a_rows.pop(k)[:],
                rhs=carry[k - 1][:],
                start=False,
                stop=True,
                skip_group_check=True,
            )
        # Stores go out in the raw (partition-major, rotated) layout:
        # out_raw[p, k, :] = y[k*T + (p-1) % T]. The host unpermutes. This
        # makes each batched SWDGE store 128 descriptors of STG*Dk*4 bytes.
        if k % STG == 0:
            osb_tiles[k // STG] = outsb_pool.tile(
                [T, STG * Dk], F32, tag="osb", name=f"osb_{k // STG}"
            )
        osb = osb_tiles[k // STG]
        dst = osb[:, (k % STG) * Dk : (k % STG + 1) * Dk]
        if k % 2 == 0:
            nc.scalar.copy(dst, ops[:])
        else:
            nc.vector.tensor_copy(dst, ops[:])
        if k % STG == STG - 1:
            g0 = k - (STG - 1)
            nc.gpsimd.dma_start(
                out=out_ap.rearrange("p (k d) -> p k d", d=Dk)[:, g0 : g0 + STG, :],
                in_=osb_tiles.pop(k // STG)[:].rearrange("p (k d) -> p k d", d=Dk),
            )

    for k in range(nt):
        if k % ch_tiles == 0:
            emit_gather(k // ch_tiles + GA)
        emit_front(k)
        if k >= QP:
            emit_back(k - QP)
    for k in range(nt - QP, nt):
        emit_back(k)


def _host_constants():
    s = np.arange(T)[:, None]
    t = np.arange(T)[None, :]
    triu = (s <= t).astype(np.float32)  # incl upper
    # column-rotated mask: out partition t' holds position (t'-1) % T
    t_rot = (t - 1) % T
    mbias = np.where(s <= t_rot, 0.0, -3e38).astype(np.float32)
    ident = np.eye(T, dtype=np.float32)
    return triu, mbias, ident


def build_nc(Lk=L, Dk=DC, n_gather_chunks=16, mm_mode="f32r"):
    nt = Lk // T
    nc = bacc.Bacc(
        "TRN2",
        target_bir_lowering=False,
        debug=False,
        enable_asserts=False,
        num_swdge_queues=2,
    )
    hid_dt = BF16 if mm_mode == "bf16g" else F32
    hid = nc.dram_tensor("hid", [Lk, Dk], hid_dt, kind="ExternalInput").ap()
    p_t = nc.dram_tensor("p_t", [T, nt], F32, kind="ExternalInput").ap()
    m16 = nc.dram_tensor("m16", [16, Lk // 16], F32, kind="ExternalInput").ap()
    triu = nc.dram_tensor("triu", [T, T], F32, kind="ExternalInput").ap()
    mbias = nc.dram_tensor("mbias", [T, T], F32, kind="ExternalInput").ap()
    ident = nc.dram_tensor("ident", [T, T], F32, kind="ExternalInput").ap()
    # raw partition-major layout: out[p, k*Dk + d] = y[k*T + (p-1)%T, d]
    out = nc.dram_tensor("out", [T, (Lk // T) * Dk], F32, kind="ExternalOutput").ap()
    with tile.TileContext(nc) as tc:
        _dechunk_tile_kernel(
            tc, out, hid, p_t, m16, triu, mbias, ident, Lk, Dk, n_gather_chunks,
            mm_mode=mm_mode,
        )
    nc.compile()
    return nc


def unpermute_out(raw, Lk=L, Dk=DC):
    """raw (T, nt*Dk) partition-major rotated -> (Lk, Dk) sequence order."""
    nt = Lk // T
    raw = raw.reshape(T, nt, Dk)
    rr = raw[(np.arange(T) + 1) % T]  # rr[q, k] = y[k*T + q]
    return np.ascontiguousarray(rr.transpose(1, 0, 2).reshape(Lk, Dk))


def make_core_inputs(hid_c, p_c, m_c, Lk=L, mm_mode="f32r"):
    """Per-core input map. hid_c (Lk, Dk) f32; p_c, m_c (Lk,) f32."""
    nt = Lk // T
    triu, mbias, ident = _host_constants()
    if mm_mode == "bf16g":
        import ml_dtypes

        hid_arr = np.ascontiguousarray(np.asarray(hid_c).astype(ml_dtypes.bfloat16))
    else:
        hid_arr = np.ascontiguousarray(hid_c, dtype=np.float32)
    return {
        "hid": hid_arr,
        "p_t": np.ascontiguousarray(
            p_c.astype(np.float32).reshape(nt, T).T
        ),  # tile-major (T, nt)
        "m16": np.ascontiguousarray(m_c.astype(np.float32).reshape(Lk // 16, 16).T),
        "triu": triu,
        "mbias": mbias,
        "ident": ident,
    }


_NC_CACHE = {}
MM_MODE = "f32r"
N_GATHER_CHUNKS = 16


def _get_nc():
    key = (L, DC, N_GATHER_CHUNKS, MM_MODE)
    if key not in _NC_CACHE:
        _NC_CACHE[key] = build_nc(L, DC, N_GATHER_CHUNKS, MM_MODE)
    return _NC_CACHE[key]


def run_cores(hidden_states, boundary_mask, boundary_prob, trace=False, **kw):
    """Shard, run on 8 NeuronCores, reassemble. Returns (out, BassKernelResults)."""
    from concourse.bass_utils import run_bass_kernel_spmd

    hidden_states = np.asarray(hidden_states, dtype=np.float32)
    boundary_mask = np.asarray(boundary_mask)
    boundary_prob = np.asarray(boundary_prob, dtype=np.float32)
    assert hidden_states.shape == (B, L, D)

    nc = _get_nc()
    in_maps = []
    for c in range(N_CORES):
        b, dh = c // 2, c % 2
        in_maps.append(
            make_core_inputs(
                hidden_states[b, :, dh * DC : (dh + 1) * DC],
                boundary_prob[b, :, 1],
                boundary_mask[b].astype(np.float32),
                mm_mode=MM_MODE,
            )
        )
    res = run_bass_kernel_spmd(nc, in_maps, list(range(N_CORES)), trace=trace, **kw)
    out = np.empty((B, L, D), dtype=np.float32)
    for c in range(N_CORES):
        b, dh = c // 2, c % 2
        out[b, :, dh * DC : (dh + 1) * DC] = unpermute_out(res.results[c]["out"])
    return out, res


def kernel(hidden_states, boundary_mask, boundary_prob):
    out, _ = run_cores(hidden_states, boundary_mask, boundary_prob, trace=False)
    return out
